# revision 6
# baseline (speedup 1.0000x reference)
"""Trainium2 Bass kernel for nn_AttentionConv2D (sparse_attention) — v5.

Math (pos-never-incremented reference quirk): per pixel i (flat h*64+w):
    att0(i) = x_i^T G x_s(i) + u0^T x_i + r^T x_s(i) + cp0      G = s Wq^T Wk
    a_p(i)  = u_p^T x_i + cp_p                                  p = 1..8
    w0      = softmax([att0, a_1..a_8])[0]   (zeroed at w=W-1 / h=H-1)
    out     = (Wv x_s(i) + bv) * w0          x_s(i) = x at pixel i+65 (0-pad)

v5 design — fully decoupled engine loops (no cross-engine round trip inside
any single engine FIFO period):
  PE : per tile one fused [V|y] MM pair (N=512) + r/u smalls + cp-mask MM
  DVE: dots (STT from PSUM, ~466ns) + d4 + recip + s3 t0-add
  ACT: 4 PLAIN V copies PSUM->SBUF (never wait w0) + exp
  gp : w0 = ex0*rd4 (mask pre-folded into cp logits as -30) + 2 in-place
       pairwise V*w0 mults on out_sb (SBUF), 1 group behind
  w=W-1 mask: cp matmul uses a [2,128] stationary (ones; -30 mask row) so
  masked pixels get logit -30 -> e^0 ~ 0 -> w0 = 0.
PSUM: 5 banks [V|y] + 3 banks logits.
"""

import os
import sys

import numpy as np

for _p in ("/opt/trn_rl_repo",):
    if _p not in sys.path:
        sys.path.append(_p)

import concourse.bass as bass
import concourse.tile as tile
from concourse import bacc, mybir
from concourse import bass_utils

F32 = mybir.dt.float32
BF16 = mybir.dt.bfloat16
F8 = mybir.dt.float8e4
AF = mybir.ActivationFunctionType
ALU = mybir.AluOpType
AX = mybir.AxisListType

B, C, H, W = 8, 256, 64, 64
HW = H * W                # 4096
A = 256
SCALE = A ** -0.5
SHIFT = W + 1             # 65
NT = 32                   # pixel tiles (128 px each)
GS = 4                    # tiles per softmax group
NG = NT // GS             # 8 groups
XCOLS = HW + 68           # padded c-major x columns
NCORES = 8
WARMUP = int(os.environ.get("KERNEL_WARMUP", "9"))

_CACHE = {}
LAST_RESULTS = None


def _build():
    nc = bacc.Bacc("TRN2", target_bir_lowering=False, debug=False)

    xcm_d = nc.dram_tensor("xcm", [128, 2, XCOLS], BF16, kind="ExternalInput").ap()
    xpm_d = nc.dram_tensor("xpm", [128, NT, C], F8, kind="ExternalInput").ap()
    # packed constants per c-chunk k (cols k*624): 0:512 wm, 512:521 u9,
    # 521:522 rr, 522:558 cp36 (row0) / slot0-indicator (row1), 560:624 pad;
    # col block 1248:1376: row0 ones, row1 maskneg (-30 at w=63 partitions)
    wpk_d = nc.dram_tensor("wpk", [128, 1376], BF16, kind="ExternalInput").ap()
    out_d = nc.dram_tensor("out", [128, NT, C], BF16, kind="ExternalOutput").ap()
    w0_d = nc.dram_tensor("w0o", [128, NT], F32, kind="ExternalOutput").ap()

    with tile.TileContext(nc) as tc:
        with (
            tc.tile_pool(name="const", bufs=1) as const,
            tc.tile_pool(name="grp", bufs=6) as grp,
            tc.tile_pool(name="outp", bufs=5) as outp,
            tc.tile_pool(name="psVY", bufs=5, space="PSUM") as psVY,
            tc.tile_pool(name="psS", bufs=3, space="PSUM") as psS,
        ):
            # ---- persistent inputs ----
            xcm2_sb = const.tile([128, 2, XCOLS], BF16, name="xcm2", tag="xcm2")
            xcm_sb = [xcm2_sb[:, k, :] for k in range(2)]
            xpm_sb = const.tile([128, NT, C], F8, name="xpm", tag="xpm")
            wpk2_sb = const.tile([128, 1376], BF16, name="wpk2", tag="wpk2")
            wpk_sb = [wpk2_sb[:, k * 624:(k + 1) * 624] for k in range(2)]
            wm_sb = [wpk_sb[k][:, 0:512] for k in range(2)]
            u_sb = [wpk_sb[k][:, 512:521] for k in range(2)]
            r_sb = [wpk_sb[k][:, 521:522] for k in range(2)]
            cp2_sb = wpk_sb[0][0:2, 522:522 + GS * 9]   # row0 cp, row1 ind0
            onem_sb = wpk2_sb[0:2, 1248:1376]           # row0 ones, row1 maskneg
            w0all = const.tile([128, NT], F32, name="w0all", tag="w0all")
            t0all = const.tile([128, NT], F32, name="t0all", tag="t0all")
            scr = const.tile([128, C], BF16, name="scr", tag="scr")

            # gpsimd queue: warmup memset FIRST
            wu_sb = const.tile([128, 512], BF16, name="wu_sb", tag="wu_sb")
            nc.gpsimd.memset(wu_sb[:], 0.0)
            zro_sb = const.tile([128, 1], F32, name="zro", tag="zro")
            nc.vector.memset(zro_sb[:], 0.0)
            # warm the ACT exp table before the pipeline needs it
            nc.scalar.activation(scr[0:1, 0:1], zro_sb[0:1, :], AF.Exp,
                                 bias=zro_sb[0:1, :])

            # ---- input streaming: ONE ring (sync), strict need-order ----
            nc.sync.dma_start(wpk2_sb[:, 0:624], wpk_d[:, 0:624])
            nc.sync.dma_start(xcm2_sb[:, :, 0:706], xcm_d[:, :, 0:706])
            nc.sync.dma_start(wpk2_sb[:, 624:1376], wpk_d[:, 624:1376])
            nc.sync.dma_start(xpm_sb[:, 0:4, :], xpm_d[:, 0:4, :])
            nc.sync.dma_start(xcm2_sb[:, :, 706:1730], xcm_d[:, :, 706:1730])
            nc.sync.dma_start(xpm_sb[:, 4:12, :], xpm_d[:, 4:12, :])
            nc.sync.dma_start(xcm2_sb[:, :, 1730:2754], xcm_d[:, :, 1730:2754])
            nc.sync.dma_start(xpm_sb[:, 12:20, :], xpm_d[:, 12:20, :])
            nc.sync.dma_start(xcm2_sb[:, :, 2754:XCOLS], xcm_d[:, :, 2754:XCOLS])
            nc.sync.dma_start(xpm_sb[:, 20:32, :], xpm_d[:, 20:32, :])

            # ---- PE warm-up: matmuls on memset data, independent of DMAs ----
            if WARMUP:
                wu_ps = psVY.tile([128, 512], F32, name="wu", tag="vy")
                nc.tensor.matmul(
                    wu_ps[:, 0:16], wu_sb[:, 0:128], wu_sb[:, 0:16],
                    start=True, stop=True,
                )
                nc.vector.tensor_scalar_add(scr[0:1, 0:1], wu_ps[0:1, 0:1], 0.0)
                for i in range(WARMUP - 1):
                    nc.tensor.matmul(
                        wu_ps[:], wu_sb[:, 0:128], wu_sb[:],
                        start=True, stop=True,
                    )

            state = {}
            vys = {}

            def front_tiles(g, jlist):
                s3 = state.get(("s3", g))
                if s3 is None:
                    s3 = psS.tile([128, GS * 9], F32, name=f"s{g}", tag="s")
                    state[("s3", g)] = s3
                for j in jlist:
                    t = g * GS + j
                    p0 = t * 128
                    vy = psVY.tile([128, 512], F32, name=f"vy{t}", tag="vy")
                    vys[t] = vy
                    xt = [xcm_sb[k][:, 1 + p0:1 + p0 + 128] for k in range(2)]
                    xs = [xcm_sb[k][:, 1 + p0 + SHIFT:1 + p0 + SHIFT + 128]
                          for k in range(2)]
                    if j == 0:
                        # cp broadcast + per-partition -30 mask on slot-0 cols
                        nc.tensor.matmul(
                            s3[:, 0:GS * 9], onem_sb, cp2_sb,
                            start=True, stop=False, skip_group_check=True,
                        )
                    # fused [V | y] GEMM + r logit, per c-chunk
                    nc.tensor.matmul(vy[:], xs[0], wm_sb[0], start=True,
                                     stop=False, skip_group_check=True)
                    nc.tensor.matmul(
                        s3[:, j * 9:j * 9 + 1], xs[0], r_sb[0],
                        start=False, stop=False, skip_group_check=True,
                    )
                    nc.tensor.matmul(vy[:], xs[1], wm_sb[1], start=False,
                                     stop=True, skip_group_check=True)
                    nc.tensor.matmul(
                        s3[:, j * 9:j * 9 + 1], xs[1], r_sb[1],
                        start=False, stop=False, skip_group_check=True,
                    )
                    # u logits
                    nc.tensor.matmul(
                        s3[:, j * 9:(j + 1) * 9], xt[0], u_sb[0],
                        start=False, stop=False, skip_group_check=True,
                    )
                    nc.tensor.matmul(
                        s3[:, j * 9:(j + 1) * 9], xt[1], u_sb[1],
                        start=False, stop=(j == GS - 1),
                        skip_group_check=True,
                    )
                    # att0 channel dot: t0all[:, t] = sum_c xpm * y
                    nc.vector.scalar_tensor_tensor(
                        scr[:], vy[:, 256:512], 1.0, xpm_sb[:, t, :],
                        ALU.mult, ALU.mult, accum_out=t0all[:, t:t + 1],
                    )

            def front_close(g):
                # s3 slot-0 += t0 for the whole group (DVE)
                s3 = state[("s3", g)]
                nc.vector.tensor_tensor(
                    s3[:, 0:GS * 9:9], s3[:, 0:GS * 9:9],
                    t0all[:, g * GS:(g + 1) * GS], ALU.add,
                )

            def mid(g):
                # ACT: plain V copies (no w0 dependency; frees PSUM fast),
                # then exp
                s3 = state.pop(("s3", g))
                out_sb = outp.tile([128, GS, C], BF16, name=f"o{g}", tag="o")
                for j in range(GS):
                    t = g * GS + j
                    nc.scalar.activation(
                        out_sb[:, j, :], vys.pop(t)[:, 0:256], AF.Identity,
                    )
                ex = grp.tile([128, GS, 9], BF16, name=f"ex{g}", tag="ex")
                nc.scalar.activation(ex[:], s3[:, 0:GS * 9], AF.Exp)
                state[g] = (ex, out_sb)

            def back_dve(g):
                # softmax scalar chain (deps resolved ~a group ago)
                ex, out_sb = state[g]
                d4 = grp.tile([128, GS], F32, name=f"d4{g}", tag="d4")
                nc.vector.tensor_reduce(d4[:], ex[:], axis=AX.X, op=ALU.add)
                rd4 = grp.tile([128, GS], F32, name=f"rd4{g}", tag="rd4")
                nc.vector.reciprocal_approx_fast(rd4[:], d4[:])
                # w0 = e0 * (1/D); mask already in the logits
                nc.gpsimd.tensor_tensor(
                    w0all[:, g * GS:(g + 1) * GS], ex[:, :, 0], rd4[:],
                    ALU.mult)
                if g == NG - 1:
                    # h = H-1 boundary: zero the last 64 pixels (tile 31)
                    nc.gpsimd.memset(w0all[64:128, NT - 1:NT], 0.0)

            def back_out(g):
                # V *= w0 in place on out_sb, pairwise; gp steady-state,
                # DVE for the final group (shorter tail)
                ex, out_sb = state.pop(g)
                last = g == NG - 1
                if last:
                    # final group: run the two pair-multiplies on gp and DVE
                    # in parallel to halve the tail chain
                    t = g * GS
                    nc.gpsimd.tensor_tensor(
                        out_sb[:, 0:2, :], out_sb[:, 0:2, :],
                        w0all[:, t:t + 2].broadcast_to((128, 2, C)), ALU.mult)
                    nc.vector.tensor_tensor(
                        out_sb[:, 2:4, :], out_sb[:, 2:4, :],
                        w0all[:, t + 2:t + 4].broadcast_to((128, 2, C)),
                        ALU.mult)
                else:
                    for jp in range(GS // 2):
                        t = g * GS + 2 * jp
                        dst = out_sb[:, 2 * jp:2 * jp + 2, :]
                        w0b = w0all[:, t:t + 2].broadcast_to((128, 2, C))
                        nc.gpsimd.tensor_tensor(dst, dst, w0b, ALU.mult)
                if last:
                    nc.sync.dma_start(
                        out_d[:, g * GS:g * GS + 2, :], out_sb[:, 0:2, :])
                    nc.sync.dma_start(
                        out_d[:, g * GS + 2:(g + 1) * GS, :], out_sb[:, 2:4, :])
                else:
                    nc.sync.dma_start(
                        out_d[:, g * GS:(g + 1) * GS, :], out_sb[:])

            for g in range(NG):
                front_tiles(g, [0, 1])
                if g >= 1:
                    back_dve(g - 1)
                front_tiles(g, [2, 3])
                front_close(g)
                if g >= 1:
                    back_out(g - 1)
                mid(g)
            back_dve(NG - 1)
            back_out(NG - 1)
            nc.sync.dma_start(w0_d[:], w0all[:])

    nc.compile()
    return nc


def _host_prep(x, Wq, bq, Wk, bk, Wv, bv):
    x = np.asarray(x, np.float32)
    Wq = np.asarray(Wq, np.float32)
    bq = np.asarray(bq, np.float32)
    Wk = np.asarray(Wk, np.float32)
    bk = np.asarray(bk, np.float32)
    Wv = np.asarray(Wv, np.float32)
    bv = np.asarray(bv, np.float32)

    # positional encoding (C, 9), matching reference._pos_encoding
    pos = np.arange(9, dtype=np.float32)[:, None]
    div = np.exp(np.arange(0, C, 2, dtype=np.float32) * (-np.log(10000.0) / C))
    pe = np.zeros((9, C), np.float32)
    pe[:, 0::2] = np.sin(pos * div)
    pe[:, 1::2] = np.cos(pos * div)
    pe = pe.T  # (C, 9)

    import ml_dtypes
    bf16 = ml_dtypes.bfloat16
    f8 = (ml_dtypes.float8_e4m3fn if hasattr(ml_dtypes, "float8_e4m3fn")
          else ml_dtypes.float8_e4m3)

    kp = Wk @ pe + bk[:, None]               # (A, 9), p = 0..8
    u9 = SCALE * (Wq.T @ kp)                 # (C, 9)
    cp = SCALE * (bq @ kp)                   # (9,)
    rr = SCALE * (Wk.T @ bq)                 # (C,)
    wm = np.concatenate([Wv.T, SCALE * (Wk.T @ Wq)], axis=1)

    wpk = np.zeros((C, 624), np.float32)
    wpk[:, 0:512] = wm
    wpk[:, 512:521] = u9
    wpk[:, 521] = rr
    wpk[0, 522:522 + GS * 9] = np.tile(cp, GS)
    ind0 = np.zeros(GS * 9, np.float32)
    ind0[0::9] = 1.0
    wpk[1, 522:522 + GS * 9] = ind0          # slot-0 indicator row

    wpk2 = np.concatenate([wpk[0:128], wpk[128:256]], axis=1)  # (128, 1248)
    onem = np.zeros((128, 128), np.float32)
    onem[0, :] = 1.0                         # ones row (partition 0)
    onem[1, 63] = -30.0                      # maskneg row: w=63 partitions
    onem[1, 127] = -30.0
    wpk2 = np.concatenate([wpk2, onem], axis=1)                # (128, 1376)
    common = {"wpk": np.ascontiguousarray(wpk2.astype(bf16))}

    in_maps = []
    for b in range(B):
        xc = x[b].reshape(C, HW)
        xcm = np.zeros((128, 2, XCOLS), bf16)
        xcm[:, 0, 1:1 + HW] = xc[0:128]
        xcm[:, 1, 1:1 + HW] = xc[128:256]
        xpm = np.ascontiguousarray(
            xc.reshape(C, NT, 128).transpose(2, 1, 0).astype(f8)
        )
        in_maps.append({"xcm": xcm, "xpm": xpm, **common})
    return in_maps


def _host_post(results, bv):
    bv = np.asarray(bv, np.float32)
    out = np.empty((B, C, H, W), np.float32)
    for b in range(B):
        o_pm = results[b]["out"].astype(np.float32)       # [128, NT, C]
        w0 = results[b]["w0o"].astype(np.float32)         # [128, NT]
        o_pm += bv[None, None, :] * w0[:, :, None]
        out[b] = o_pm.transpose(2, 1, 0).reshape(C, H, W)
    return out


def kernel(x, Wq, bq, Wk, bk, Wv, bv):
    global LAST_RESULTS
    if "nc" not in _CACHE:
        _CACHE["nc"] = _build()
    nc = _CACHE["nc"]

    in_maps = _host_prep(x, Wq, bq, Wk, bk, Wv, bv)
    res = bass_utils.run_bass_kernel_spmd(
        nc, in_maps, core_ids=list(range(NCORES)),
        trace=bool(os.environ.get("KERNEL_TRACE")),
    )
    LAST_RESULTS = res
    return _host_post(res.results, bv)
